# revision 25
# baseline (speedup 1.0000x reference)
"""Trainium2 Bass kernel for nn_Attention_78048145703090 (sparse_attention).

Math: the reference's [N,N] attention is rank-1 structured: every logit row n
is w_n * s where s[m] = scale * (q_center . k_m) is one shared score vector
per sample and w_n = exp(1 - dist_n) depends only on grid distance; only
U=457 distinct w values exist. Instead of materializing E'[m,u] =
exp(w_u * s_m) with 1.87M on-chip exps (ACT-bound, ~26us in a previous
version), this kernel uses a polynomial factorization: with sn = s / A_S in
[-1, 1],

    E'[m, u] ~= sum_k V[k, u] * sn_m^k     (degree-12 fit per u, host-fit)

so the contraction YT = [x|1]^T E' = ([x|1]^T P) V where P[m, k] = sn_m^k is
built with 4 log-depth DVE multiplies. The moments A = [x|1]^T P are 32 tiny
PE matmuls; YT = A^T V is one f32 matmul. Zero exps on-chip; the exp lives in
the host-precomputed V (Chebyshev-projected, converted to monomial basis).

Downstream algebra is folded hard:
  - wv/wp collapse into one weight CW = [wv@wp ; bv@wp + bp] applied to
    [Y|den] (biases ride the den row), giving qT with p = qT * r, r = 1/den.
  - r is computed per-partition: den row is PE-transposed to columns, then
    ACT Ln + Exp(-x) (reciprocal via exp(-ln), since DVE reciprocal is
    8 cyc/elem and ACT Reciprocal is banned).
  - the u->n expansion (4096 rows gathered from 457 uniques) is a one-hot
    bf16 matmul against a host-built [P, JC, N] gather matrix streamed from
    HBM starting at t=0 (fully hidden behind the front compute); r-scaling
    is fused into the per-chunk post-transpose copies (tensor_scalar_mul).
    (A dma_gather row-gather variant measured ~9.5us/1024 rows of Q7
    descriptor generation - far slower than the one-hot stream.)

Sharding: data-parallel over B=8 across the 8 cores (one sample per core);
each core holds the full 64x64 weights.
"""

import os
import sys

sys.path.insert(0, "/opt/trn_rl_repo")

import numpy as np

import concourse.bacc as bacc
import concourse.mybir as mybir
import concourse.tile as tile
from concourse import masks
from concourse.tile_rust import add_dep_helper


def _install_profile_hook():
    """This image's antenv lacks axon_hooks; reconstruct it so
    run_bass_kernel_spmd(trace=True) can capture NTFF profiles. No-op for
    normal (untraced) runs."""
    import types

    try:
        import antenv.axon_hooks  # noqa: F401

        return
    except ImportError:
        pass
    try:
        import antenv

        m = types.ModuleType("antenv.axon_hooks")
        state = {"hook": None}
        m.set_axon_ntff_profile_hook = lambda h: state.__setitem__("hook", h)
        m.get_axon_ntff_profile_hook = lambda: state["hook"]
        sys.modules["antenv.axon_hooks"] = m
        antenv.axon_hooks = m
        from trn_agent_boot.trn_boot import _ntff_profile_via_ctypes

        m.set_axon_ntff_profile_hook(
            _ntff_profile_via_ctypes("/opt/axon/libaxon_pjrt.so")
        )
    except Exception:
        pass


_install_profile_hook()

from concourse.bass_utils import run_bass_kernel_spmd

B, H, W, C = 8, 64, 64, 64
N = H * W  # 4096
P = 128
NCH = N // P  # 32
HH = NCH // 2  # 16
CENTER = (H // 2) * W + (W // 2)  # 2080
C_CH = CENTER % NCH  # chunk (inner index) holding the center row: 0
C_PCOL = CENTER // NCH  # partition/column of the center row: 65
SCALE = float(C) ** -0.5
F32 = mybir.dt.float32
BF16 = mybir.dt.bfloat16

KDEG = int(os.environ.get("K_DEG", "12"))  # polynomial degree (4 DVE muls)
KP1 = KDEG + 1
A_S = 18.0  # |s| bound; sn = s / A_S in [-1, 1]
NS = 8  # output column slices for the one-hot expansion

# ---- compile-time constants derived from the distance grid ----
_yy, _xx = np.mgrid[0:H, 0:W]
_d2 = ((_yy - H // 2) ** 2 + (_xx - W // 2) ** 2).reshape(-1)  # [N] int
_uniq_d2, _g = np.unique(_d2, return_inverse=True)
U = len(_uniq_d2)  # 457
JC = (U + P - 1) // P  # 4 chunks: 128,128,128,73
CS = [min(P, U - jc * P) for jc in range(JC)]

# w_u with the attention scale folded in; polynomial arg is (w_u*A_S)*sn
_w_u = np.exp(1.0 - np.sqrt(_uniq_d2.astype(np.float64))) * SCALE


def _build_V():
    from numpy.polynomial import chebyshev as Ch

    npts = 4 * KDEG + 16
    yg = np.cos(np.pi * (np.arange(npts) + 0.5) / npts)
    F = np.exp(yg[:, None] * (_w_u * A_S)[None, :])  # [npts, U] f64
    Cc = Ch.chebfit(yg, F, KDEG)  # [KP1, U] chebyshev coeffs
    T = np.zeros((KP1, KP1))
    for k in range(KP1):
        e = np.zeros(KP1)
        e[k] = 1.0
        pk = Ch.cheb2poly(e)
        T[: len(pk), k] = pk
    return np.ascontiguousarray((T @ Cc).astype(np.float32))  # [KP1, U] mono


V_MAT = _build_V()

# one-hot gather matrix (bf16, exact), packed [P, JC, N]; columns permuted so
# each transposed 128-col strip is {p*32+s : p} for one s, giving an
# 8KB-contiguous store per partition after the final transposes
import ml_dtypes

GT = np.zeros((P, JC, N), ml_dtypes.bfloat16)
GT[_g % P, _g // P, np.arange(N)] = 1.0
GT = np.ascontiguousarray(
    GT.reshape(P, JC, P, NCH).transpose(0, 1, 3, 2).reshape(P, JC, N)
)


def _bc(sl, reps):
    """Broadcast the innermost (size-1) dim of a sliced AP to `reps`."""
    return type(sl)(
        tensor=sl.tensor, offset=sl.offset, ap=[*sl.ap[:-1], [0, reps]]
    )


def build_nc():
    nc = bacc.Bacc("TRN2", target_bir_lowering=False, debug=False, num_devices=B)
    xb = nc.dram_tensor("xb", [N, C], F32, kind="ExternalInput")
    wqk1 = nc.dram_tensor("wqk1", [C + 1, C], F32, kind="ExternalInput")
    wvp1 = nc.dram_tensor("wvp1", [C + 1, C + 1], F32, kind="ExternalInput")
    vmat = nc.dram_tensor("vmat", [KP1, U], F32, kind="ExternalInput")
    gt = nc.dram_tensor("gt", [P, JC, N], BF16, kind="ExternalInput")
    out = nc.dram_tensor("out", [N, C], F32, kind="ExternalOutput")

    xv = xb.ap().rearrange("(p i) c -> p i c", p=P)
    ov = out.ap().rearrange("(p s) c -> p s c", p=P)

    with tile.TileContext(nc) as tc:
        with (
            tc.tile_pool(name="consts", bufs=1) as consts,
            tc.tile_pool(name="sb", bufs=1) as sb,
            tc.tile_pool(name="obt_sb_pool", bufs=3) as obt_sb_pool,
            tc.tile_pool(name="ps_t", bufs=2, space="PSUM") as ps_t,
            tc.tile_pool(name="ps_a", bufs=1, space="PSUM") as ps_a,
            tc.tile_pool(name="ps_small", bufs=2, space="PSUM") as ps_small,
            tc.tile_pool(name="ps_ob", bufs=2, space="PSUM") as ps_ob,
            tc.tile_pool(name="ps_w", bufs=1, space="PSUM") as ps_w,
        ):
            ident = consts.tile([P, P], F32)
            masks.make_identity(nc, ident[:])
            identb = consts.tile([P, P], BF16)
            masks.make_identity(nc, identb[:])
            ones_row = consts.tile([1, P], F32)
            nc.vector.memset(ones_row[:], 1.0)

            # preload the ACT copy-table set while DMAs run
            actwarm = sb.tile([1, 2], F32)
            nc.scalar.copy(out=actwarm[:, 0:1], in_=ones_row[:, 0:1])

            # ---- DMA order on the sync ring (FIFO): x chunk0, wqk1, x rest,
            # wvp1, vmat, gt (4.7MB stream fully hidden behind compute)
            # x + gt on the sync HWDGE ring; small weights on the scalar
            # HWDGE ring so they don't delay x (rings drain FIFO)
            x_sb = sb.tile([P, NCH, C], F32)
            xc0 = nc.sync.dma_start(out=x_sb[:, 0:1, :], in_=xv[:, 0:1, :])
            xha = nc.sync.dma_start(out=x_sb[:, 1:HH, :], in_=xv[:, 1:HH, :])
            xhb = nc.sync.dma_start(
                out=x_sb[:, HH:NCH, :], in_=xv[:, HH:NCH, :]
            )
            gt_sb = consts.tile([P, JC, N], BF16)
            gt_dma = nc.sync.dma_start(out=gt_sb[:], in_=gt[:])
            wqk1_sb = consts.tile([C + 1, C], F32)
            wq_dma = nc.scalar.dma_start(out=wqk1_sb[:], in_=wqk1[:])
            wvp1_sb = consts.tile([C + 1, C + 1], F32)
            wvp_dma = nc.scalar.dma_start(out=wvp1_sb[:], in_=wvp1[:])
            v_sb = consts.tile([KP1, U], F32)
            v_dma = nc.scalar.dma_start(out=v_sb[:], in_=vmat[:])
            for a, b_ in (
                (xha, xc0),
                (xhb, xha),
                (gt_dma, xhb),
                (wvp_dma, wq_dma),
                (v_dma, wvp_dma),
            ):
                add_dep_helper(a.ins, b_.ins, sync=False, reason="dma order")

            # bf16 ones-column copy of x, per half, on the scalar engine
            x1f = sb.tile([P, NCH, C + 1], BF16)
            nc.vector.memset(x1f[:, :, C : C + 1], 1.0)
            for h in range(2):
                i0 = h * HH
                nc.scalar.copy(
                    out=x1f[:, i0 : i0 + HH, 0:C], in_=x_sb[:, i0 : i0 + HH, :]
                )

            # q_center: transpose the center chunk, take the center column
            qcr_sb = sb.tile([C + 1, 1], F32)
            nc.vector.memset(qcr_sb[:], 1.0)
            xrow_ps = ps_small.tile([C, P], F32, tag="m")
            nc.tensor.transpose(
                out=xrow_ps[:], in_=x_sb[:, C_CH, :], identity=ident[:]
            )
            nc.vector.tensor_copy(
                out=qcr_sb[0:C, :], in_=xrow_ps[:, C_PCOL : C_PCOL + 1]
            )
            # u_row = qcr^T [wq.T wk ; bq wk] (pre-scaled by 1/A_S on host)
            ur_ps = ps_small.tile([1, C], F32, tag="m")
            nc.tensor.matmul(ur_ps[:], qcr_sb[:], wqk1_sb[:], start=True, stop=True)
            ur_sb = sb.tile([1, C], F32)
            nc.vector.tensor_copy(out=ur_sb[:], in_=ur_ps[:])
            ubc_ps = ps_small.tile([P, C], F32, tag="m")
            nc.tensor.matmul(ubc_ps[:], ones_row[:], ur_sb[:], start=True, stop=True)
            ubc_sb = sb.tile([P, C], F32)
            nc.vector.tensor_copy(out=ubc_sb[:], in_=ubc_ps[:])

            # sn[m] = x[m, :] . u  (already 1/A_S-scaled): mul + reduce per half
            s_col = sb.tile([P, NCH], F32)
            xu = [sb.tile([P, HH, C], F32, name=f"xu{h}") for h in range(2)]
            ubc_ap = ubc_sb[:]
            ubc_h = type(ubc_ap)(
                tensor=ubc_ap.tensor,
                offset=ubc_ap.offset,
                ap=[ubc_ap.ap[0], [0, HH], ubc_ap.ap[1]],
            )
            for h in range(2):
                i0 = h * HH
                nc.vector.tensor_mul(xu[h][:], x_sb[:, i0 : i0 + HH, :], ubc_h)
                nc.vector.reduce_sum(
                    out=s_col[:, i0 : i0 + HH],
                    in_=xu[h][:],
                    axis=mybir.AxisListType.X,
                )

            # PE warm-up burst: ~6us of back-to-back matmuls over x while the
            # DVE computes sn, so the HAM clock gate opens (1.2 -> 2.4 GHz)
            # before the moment/expansion matmuls. Results are never read.
            warm_ps = ps_w.tile([C, 512], F32, tag="warm")
            x_ap = x_sb[:]
            x_flat = type(x_ap)(
                tensor=x_ap.tensor, offset=x_ap.offset, ap=[x_ap.ap[0], [1, 512]]
            )
            for wi in range(3):
                nc.tensor.matmul(
                    warm_ps[:],
                    x_sb[:, 0, :],
                    x_flat,
                    start=(wi == 0),
                    stop=(wi == 2),
                )



            # monomial basis S[:, i, k] = sn^k via log-depth squaring (f32)
            S = sb.tile([P, NCH, KP1], F32)
            nc.vector.memset(S[:, :, 0:1], 1.0)
            nc.vector.tensor_copy(out=S[:, :, 1:2], in_=s_col[:])
            nc.vector.tensor_mul(S[:, :, 2:3], S[:, :, 1:2], S[:, :, 1:2])
            nc.vector.tensor_mul(S[:, :, 3:5], S[:, :, 1:3], _bc(S[:, :, 2:3], 2))
            nc.vector.tensor_mul(S[:, :, 5:9], S[:, :, 1:5], _bc(S[:, :, 4:5], 4))
            nc.vector.tensor_mul(
                S[:, :, 9:KP1], S[:, :, 5 : KP1 - 4], _bc(S[:, :, 4:5], 4)
            )
            Sb = sb.tile([P, NCH, KP1], BF16)
            nc.vector.tensor_copy(out=Sb[:], in_=S[:])

            # moments A^T[c1, k] = sum_m [x|1] sn^k: 32 accumulating matmuls
            at_ps = ps_a.tile([C + 1, KP1], F32)
            for i in range(NCH):
                nc.tensor.matmul(
                    at_ps[:],
                    x1f[:, i, :],
                    Sb[:, i, :],
                    start=(i == 0),
                    stop=(i == NCH - 1),
                )
            at_sb = sb.tile([C + 1, KP1], F32)
            nc.vector.tensor_copy(out=at_sb[:], in_=at_ps[:])

            # fold wv/wp/biases: acw = A^T^T [CW|e64] -> [KP1, 65], then
            # [qT ; den] = acw^T V in one f32 matmul [65, U]
            acw_ps = ps_small.tile([KP1, C + 1], F32, tag="m")
            nc.tensor.matmul(acw_ps[:], at_sb[:], wvp1_sb[:], start=True, stop=True)
            acw_sb = sb.tile([KP1, C + 1], F32)
            nc.vector.tensor_copy(out=acw_sb[:], in_=acw_ps[:])
            qtd_ps = ps_small.tile([C + 1, U], F32, tag="m")
            nc.tensor.matmul(qtd_ps[:], acw_sb[:], v_sb[:], start=True, stop=True)
            qtd_sb = sb.tile([C + 1, U], F32)
            nc.scalar.copy(out=qtd_sb[:], in_=qtd_ps[:])

            # den row -> columns (4 tiny PE transposes), r = 1/den on DVE
            # (4 elems/lane: the iterative divide is cheap in this layout)
            denT_ps = ps_small.tile([P, JC], F32, tag="m")
            for jc in range(JC):
                cs = CS[jc]
                nc.tensor.transpose(
                    out=denT_ps[0:cs, jc : jc + 1],
                    in_=qtd_sb[C : C + 1, jc * P : jc * P + cs],
                    identity=ident[C : C + 1, C : C + 1],
                )
            rT_sb = sb.tile([P, JC], F32)
            nc.vector.reciprocal(out=rT_sb[:], in_=denT_ps[:])

            # transpose chunks to [u, c]; fuse the r scaling + bf16 cast
            p_sb = sb.tile([P, JC, C], BF16)
            for jc in range(JC):
                cs = CS[jc]
                tp2 = ps_t.tile([P, C], F32, tag="tb")
                nc.tensor.transpose(
                    out=tp2[0:cs, :],
                    in_=qtd_sb[0:C, jc * P : jc * P + cs],
                    identity=ident[0:C, 0:C],
                )
                nc.vector.tensor_scalar_mul(
                    p_sb[0:cs, jc, :], tp2[0:cs, :], rT_sb[0:cs, jc : jc + 1]
                )

            # expand unique rows to all 4096 positions: out^T computed TWO
            # slices at a time via column-tiled concurrent matmuls (col
            # groups 0:64 and 64:128 of the PE array run in parallel), then
            # transpose each 128-col strip back to [n, c], store f32
            SL = N // NS  # 512 permuted columns = 4 s-slots per slice
            SK = SL // P  # 4
            o_big = sb.tile([P, NCH, C], F32)
            for q in range(NS // 2):
                obT = ps_ob.tile([P, SL], F32)
                for half in range(2):
                    ns = 2 * q + half
                    for jc in range(JC):
                        cs = CS[jc]
                        nc.tensor.matmul(
                            obT[half * C : half * C + C, :],
                            p_sb[0:cs, jc, :],
                            gt_sb[0:cs, jc, ns * SL : (ns + 1) * SL],
                            start=(jc == 0),
                            stop=(jc == JC - 1),
                            tile_position=(0, half * C),
                        )
                obT_sb = obt_sb_pool.tile([P, SL], BF16)
                if q % 2 == 0:
                    nc.vector.tensor_copy(out=obT_sb[:], in_=obT[:])
                else:
                    nc.scalar.copy(out=obT_sb[:], in_=obT[:])
                for k in range(SK):
                    for half in range(2):
                        ns = 2 * q + half
                        h0 = half * C
                        s_slot = ns * SK + k
                        on_ps = ps_t.tile([P, C], BF16, tag="tb")
                        nc.tensor.transpose(
                            out=on_ps[:],
                            in_=obT_sb[h0 : h0 + C, k * P : (k + 1) * P],
                            identity=identb[h0 : h0 + C, h0 : h0 + C],
                        )
                        if (k + half) % 2 == 0:
                            nc.vector.tensor_copy(
                                out=o_big[:, s_slot, :], in_=on_ps[:]
                            )
                        else:
                            nc.scalar.copy(out=o_big[:, s_slot, :], in_=on_ps[:])
                s0 = 2 * q * SK
                nc.sync.dma_start(
                    out=ov[:, s0 : s0 + 2 * SK, :],
                    in_=o_big[:, s0 : s0 + 2 * SK, :],
                )

    nc.compile()
    return nc


_nc_cache = None


def _get_nc():
    global _nc_cache
    if _nc_cache is None:
        _nc_cache = build_nc()
    return _nc_cache


def make_in_maps(x, wq, bq, wk, bk, wv, bv, wp, bp):
    f = lambda a: np.ascontiguousarray(np.asarray(a, dtype=np.float32))
    x = f(x)
    wv_, wp_, bv_, bp_ = f(wv), f(wp), f(bv), f(bp)
    shared = {
        "wqk1": (
            np.concatenate([f(wq).T @ f(wk), (f(bq) @ f(wk))[None, :]], 0)
            * np.float32(1.0 / A_S)
        ).astype(np.float32),
        "wvp1": np.concatenate(
            [
                np.concatenate(
                    [(wp_ @ wv_).T, (wp_ @ bv_ + bp_)[None, :]], 0
                ),
                np.eye(C + 1, dtype=np.float32)[:, C : C + 1],
            ],
            1,
        ).astype(np.float32),
        "vmat": V_MAT,
        "gt": GT,
    }
    shared = {k: np.ascontiguousarray(v) for k, v in shared.items()}
    return [
        {"xb": np.ascontiguousarray(x[b].reshape(N, C)), **shared}
        for b in range(B)
    ]


def kernel_with_results(trace=False, **inputs):
    in_maps = make_in_maps(**inputs)
    nc = _get_nc()
    res = run_bass_kernel_spmd(nc, in_maps, core_ids=list(range(B)), trace=trace)
    out = np.stack([r["out"] for r in res.results], 0).reshape(B, H, W, C)
    return out, res


def kernel(**inputs):
    out, _ = kernel_with_results(**inputs)
    return out


# revision 26
# speedup vs baseline: 1.2169x; 1.2169x over previous
"""Trainium2 Bass kernel for nn_Attention_78048145703090 (sparse_attention).

Math: the reference's [N,N] attention is rank-1 structured: every logit row n
is W_n * s where s[m] = scale * (q_center . k_m) is one shared score vector
per sample and W_n = exp(1 - dist_n) * scale depends only on grid distance.
Instead of materializing E'[m,n] = exp(W_n * s_m) with 1.87M on-chip exps
(ACT-bound, ~26us in the original), the kernel uses a bivariate polynomial
fit (host-side least squares, f64):

    exp(W_n * A_S * y) ~= sum_{j<=J, k<=K} B[j,k] * W_n^j * y^k,  y = s/A_S

With P[m,k] = sn_m^k built by 4 log-depth DVE multiplies, the whole attention
collapses to small dense algebra:

    A^T  = [x|1]^T P                  (32 tiny PE matmuls, bf16)
    acw  = A^T^T [wv@wp ; bv@wp+bp | e]   (fold V+proj+biases; biases ride
                                           the ones-moment row)   [K+1, 65]
    G    = B acw                      [J+1, 65]  (tiny f32 matmul)
    num[n, :] = sum_j W_n^j G[j, 0:64]   -> 32 matmuls vs a host-constant
                                            f32 Vandermonde in W_n
    den[n]    = sum_j W_n^j G[j, 64]     -> one DVE mul+reduce+reciprocal
    out[n, :] = num[n, :] / den[n]       -> fused per-chunk scalar scaling

No exps, no softmax, no [N,N] or [N,U] intermediates, no transposes of the
output, and no gather: the Vandermonde stationary (230KB) replaces the 4MiB
one-hot matrix a previous version streamed. The W-direction evaluation must
stay f32 (exponential dynamic range -> bf16 cancellation fails); x and the
sn-powers are bf16 (validated 3.1e-4 end-to-end).

Sharding: data-parallel over B=8 across the 8 cores (one sample per core);
each core holds the full 64x64 weights.
"""

import os
import sys

sys.path.insert(0, "/opt/trn_rl_repo")

import numpy as np

import concourse.bacc as bacc
import concourse.mybir as mybir
import concourse.tile as tile
from concourse import masks
from concourse.tile_rust import add_dep_helper


def _install_profile_hook():
    """This image's antenv lacks axon_hooks; reconstruct it so
    run_bass_kernel_spmd(trace=True) can capture NTFF profiles. No-op for
    normal (untraced) runs."""
    import types

    try:
        import antenv.axon_hooks  # noqa: F401

        return
    except ImportError:
        pass
    try:
        import antenv

        m = types.ModuleType("antenv.axon_hooks")
        state = {"hook": None}
        m.set_axon_ntff_profile_hook = lambda h: state.__setitem__("hook", h)
        m.get_axon_ntff_profile_hook = lambda: state["hook"]
        sys.modules["antenv.axon_hooks"] = m
        antenv.axon_hooks = m
        from trn_agent_boot.trn_boot import _ntff_profile_via_ctypes

        m.set_axon_ntff_profile_hook(
            _ntff_profile_via_ctypes("/opt/axon/libaxon_pjrt.so")
        )
    except Exception:
        pass


_install_profile_hook()

from concourse.bass_utils import run_bass_kernel_spmd

B, H, W, C = 8, 64, 64, 64
N = H * W  # 4096
P = 128
NCH = N // P  # 32
HH = NCH // 2  # 16
CENTER = (H // 2) * W + (W // 2)  # 2080
C_CH = CENTER % NCH  # chunk (inner index) holding the center row: 0
C_PCOL = CENTER // NCH  # partition/column of the center row: 65
SCALE = float(C) ** -0.5
F32 = mybir.dt.float32
BF16 = mybir.dt.bfloat16

KDEG = int(os.environ.get("K_DEG", "12"))  # sn-polynomial degree
JDEG = int(os.environ.get("J_DEG", "12"))  # W-polynomial degree
KP1 = KDEG + 1
JP1 = JDEG + 1
A_S = 18.0  # |s| bound; sn = s / A_S in [-1, 1]

# ---- compile-time constants derived from the distance grid ----
_yy, _xx = np.mgrid[0:H, 0:W]
_d2 = ((_yy - H // 2) ** 2 + (_xx - W // 2) ** 2).reshape(-1)  # [N] int
_Wn = np.exp(1.0 - np.sqrt(_d2.astype(np.float64))) * SCALE  # [N] weights


def _build_B():
    """Least-squares bivariate fit exp(w*A_S*y) ~= sum B[j,k] w^j y^k over
    the actual unique w values x chebyshev y-grid, in f64."""
    wg = np.unique(_Wn)
    npts = 4 * KDEG + 16
    yg = np.cos(np.pi * (np.arange(npts) + 0.5) / npts)
    WW, YY = np.meshgrid(wg, yg, indexing="ij")
    F = np.exp(WW * A_S * YY).reshape(-1)
    basis = np.stack(
        [
            (WW.reshape(-1) ** j) * (YY.reshape(-1) ** k)
            for j in range(JP1)
            for k in range(KP1)
        ],
        1,
    )
    coef, *_ = np.linalg.lstsq(basis, F, rcond=None)
    return coef.reshape(JP1, KP1)  # B[j, k]


_B = _build_B()
BMAT = np.ascontiguousarray(_B.T.astype(np.float32))  # [KP1, JP1] stationary

# Vandermonde in W_n, f32. Stationary layout: column p of chunk s is
# n = p*32 + s (matches the [p, s, c] output staging); DVE layout wpj[p,s,j].
_wv = np.stack([_Wn**j for j in range(JP1)], 0).astype(np.float32)  # [JP1, N]
_n_of = (np.arange(P)[None, :] * NCH + np.arange(NCH)[:, None]).reshape(-1)
WVS = np.ascontiguousarray(_wv[:, _n_of])  # [JP1, 32*128], chunk-s blocks
WPJ = np.ascontiguousarray(
    _wv.T.reshape(P, NCH, JP1).astype(np.float32)
)  # [p, s, j] for n = p*32+s


def _bc(sl, reps):
    """Broadcast the innermost (size-1) dim of a sliced AP to `reps`."""
    return type(sl)(
        tensor=sl.tensor, offset=sl.offset, ap=[*sl.ap[:-1], [0, reps]]
    )


def build_nc():
    nc = bacc.Bacc("TRN2", target_bir_lowering=False, debug=False, num_devices=B)
    xb = nc.dram_tensor("xb", [N, C], F32, kind="ExternalInput")
    wqk1 = nc.dram_tensor("wqk1", [C + 1, C], F32, kind="ExternalInput")
    wvp1 = nc.dram_tensor("wvp1", [C + 1, C + 1], F32, kind="ExternalInput")
    bmat = nc.dram_tensor("bmat", [KP1, JP1], F32, kind="ExternalInput")
    wvs = nc.dram_tensor("wvs", [JP1, N], F32, kind="ExternalInput")
    wpj = nc.dram_tensor("wpj", [P, NCH * JP1], F32, kind="ExternalInput")
    out = nc.dram_tensor("out", [N, C], F32, kind="ExternalOutput")

    xv = xb.ap().rearrange("(p i) c -> p i c", p=P)
    ov = out.ap().rearrange("(p s) c -> p s c", p=P)

    with tile.TileContext(nc) as tc:
        with (
            tc.tile_pool(name="consts", bufs=1) as consts,
            tc.tile_pool(name="sb", bufs=1) as sb,
            tc.tile_pool(name="ps_t", bufs=2, space="PSUM") as ps_t,
            tc.tile_pool(name="ps_a", bufs=1, space="PSUM") as ps_a,
            tc.tile_pool(name="ps_small", bufs=2, space="PSUM") as ps_small,
            tc.tile_pool(name="ps_num", bufs=4, space="PSUM") as ps_num,
            tc.tile_pool(name="ps_w", bufs=1, space="PSUM") as ps_w,
        ):
            ident = consts.tile([P, P], F32)
            masks.make_identity(nc, ident[:])
            ones_row = consts.tile([1, P], F32)
            nc.vector.memset(ones_row[:], 1.0)

            # preload the ACT copy-table set while DMAs run
            actwarm = sb.tile([1, 2], F32)
            nc.scalar.copy(out=actwarm[:, 0:1], in_=ones_row[:, 0:1])

            # x on the sync HWDGE ring; Vandermonde consts follow it; small
            # weights on the scalar HWDGE ring (rings drain FIFO)
            x_sb = sb.tile([P, NCH, C], F32)
            xc0 = nc.sync.dma_start(out=x_sb[:, 0:1, :], in_=xv[:, 0:1, :])
            xha = nc.sync.dma_start(out=x_sb[:, 1:HH, :], in_=xv[:, 1:HH, :])
            xhb = nc.sync.dma_start(
                out=x_sb[:, HH:NCH, :], in_=xv[:, HH:NCH, :]
            )
            wvs_sb = consts.tile([JP1, N], F32)
            wvs_dma = nc.sync.dma_start(out=wvs_sb[:], in_=wvs[:])
            wpj_sb = consts.tile([P, NCH, JP1], F32)
            wpj_dma = nc.sync.dma_start(
                out=wpj_sb[:],
                in_=wpj.ap().rearrange("p (s j) -> p s j", s=NCH),
            )
            wqk1_sb = consts.tile([C + 1, C], F32)
            wq_dma = nc.scalar.dma_start(out=wqk1_sb[:], in_=wqk1[:])
            wvp1_sb = consts.tile([C + 1, C + 1], F32)
            wvp_dma = nc.scalar.dma_start(out=wvp1_sb[:], in_=wvp1[:])
            bmat_sb = consts.tile([KP1, JP1], F32)
            bmat_dma = nc.scalar.dma_start(out=bmat_sb[:], in_=bmat[:])
            for a, b_ in (
                (xha, xc0),
                (xhb, xha),
                (wvs_dma, xhb),
                (wpj_dma, wvs_dma),
                (wvp_dma, wq_dma),
                (bmat_dma, wvp_dma),
            ):
                add_dep_helper(a.ins, b_.ins, sync=False, reason="dma order")

            # bf16 ones-column copy of x, per half, on the scalar engine
            x1f = sb.tile([P, NCH, C + 1], BF16)
            nc.vector.memset(x1f[:, :, C : C + 1], 1.0)
            for h in range(2):
                i0 = h * HH
                nc.scalar.copy(
                    out=x1f[:, i0 : i0 + HH, 0:C], in_=x_sb[:, i0 : i0 + HH, :]
                )

            # q_center: transpose the center chunk, take the center column
            qcr_sb = sb.tile([C + 1, 1], F32)
            nc.vector.memset(qcr_sb[:], 1.0)
            xrow_ps = ps_small.tile([C, P], F32, tag="m")
            nc.tensor.transpose(
                out=xrow_ps[:], in_=x_sb[:, C_CH, :], identity=ident[:]
            )
            nc.vector.tensor_copy(
                out=qcr_sb[0:C, :], in_=xrow_ps[:, C_PCOL : C_PCOL + 1]
            )
            # u_row = qcr^T [wq.T wk ; bq wk] (pre-scaled by 1/A_S on host)
            ur_ps = ps_small.tile([1, C], F32, tag="m")
            nc.tensor.matmul(ur_ps[:], qcr_sb[:], wqk1_sb[:], start=True, stop=True)
            ur_sb = sb.tile([1, C], F32)
            nc.vector.tensor_copy(out=ur_sb[:], in_=ur_ps[:])
            ubc_ps = ps_small.tile([P, C], F32, tag="m")
            nc.tensor.matmul(ubc_ps[:], ones_row[:], ur_sb[:], start=True, stop=True)
            ubc_sb = sb.tile([P, C], F32)
            nc.vector.tensor_copy(out=ubc_sb[:], in_=ubc_ps[:])

            # sn[m] = x[m, :] . u  (already 1/A_S-scaled): mul + reduce per half
            s_col = sb.tile([P, NCH], F32)
            xu = [sb.tile([P, HH, C], F32, name=f"xu{h}") for h in range(2)]
            ubc_ap = ubc_sb[:]
            ubc_h = type(ubc_ap)(
                tensor=ubc_ap.tensor,
                offset=ubc_ap.offset,
                ap=[ubc_ap.ap[0], [0, HH], ubc_ap.ap[1]],
            )
            for h in range(2):
                i0 = h * HH
                nc.vector.tensor_mul(xu[h][:], x_sb[:, i0 : i0 + HH, :], ubc_h)
                nc.vector.reduce_sum(
                    out=s_col[:, i0 : i0 + HH],
                    in_=xu[h][:],
                    axis=mybir.AxisListType.X,
                )

            # PE warm-up burst: back-to-back matmuls over x while the DVE
            # computes sn, so the HAM clock gate opens (1.2 -> 2.4 GHz)
            # before the moment matmuls. Results are never read.
            warm_ps = ps_w.tile([C, 512], F32, tag="warm")
            x_ap = x_sb[:]
            x_flat = type(x_ap)(
                tensor=x_ap.tensor, offset=x_ap.offset, ap=[x_ap.ap[0], [1, 512]]
            )
            for wi in range(3):
                nc.tensor.matmul(
                    warm_ps[:],
                    x_sb[:, 0, :],
                    x_flat,
                    start=(wi == 0),
                    stop=(wi == 2),
                )

            # monomial basis S[:, i, k] = sn^k via log-depth squaring (f32)
            S = sb.tile([P, NCH, KP1], F32)
            nc.vector.memset(S[:, :, 0:1], 1.0)
            nc.vector.tensor_copy(out=S[:, :, 1:2], in_=s_col[:])
            nc.vector.tensor_mul(S[:, :, 2:3], S[:, :, 1:2], S[:, :, 1:2])
            nc.vector.tensor_mul(S[:, :, 3:5], S[:, :, 1:3], _bc(S[:, :, 2:3], 2))
            nc.vector.tensor_mul(S[:, :, 5:9], S[:, :, 1:5], _bc(S[:, :, 4:5], 4))
            nc.vector.tensor_mul(
                S[:, :, 9:KP1], S[:, :, 5 : KP1 - 4], _bc(S[:, :, 4:5], 4)
            )
            Sb = sb.tile([P, NCH, KP1], BF16)
            nc.vector.tensor_copy(out=Sb[:], in_=S[:])

            # moments A^T[c1, k] = sum_m [x|1] sn^k: 32 accumulating matmuls
            at_ps = ps_a.tile([C + 1, KP1], F32)
            for i in range(NCH):
                nc.tensor.matmul(
                    at_ps[:],
                    x1f[:, i, :],
                    Sb[:, i, :],
                    start=(i == 0),
                    stop=(i == NCH - 1),
                )
            at_sb = sb.tile([C + 1, KP1], F32)
            nc.vector.tensor_copy(out=at_sb[:], in_=at_ps[:])

            # acw = A^T^T [CW|e64] (fold wv/wp/biases), then G = B acw
            acw_ps = ps_small.tile([KP1, C + 1], F32, tag="m")
            nc.tensor.matmul(acw_ps[:], at_sb[:], wvp1_sb[:], start=True, stop=True)
            acw_sb = sb.tile([KP1, C + 1], F32)
            nc.vector.tensor_copy(out=acw_sb[:], in_=acw_ps[:])
            g_ps = ps_small.tile([JP1, C + 1], F32, tag="m")
            nc.tensor.matmul(g_ps[:], bmat_sb[:], acw_sb[:], start=True, stop=True)
            g_sb = sb.tile([JP1, C + 1], F32)
            nc.vector.tensor_copy(out=g_sb[:], in_=g_ps[:])

            # den coefficients: transpose G col 64 -> row, broadcast across
            # partitions, then den/reciprocal per position on the DVE
            gdT_ps = ps_small.tile([1, JP1], F32, tag="m")
            nc.tensor.transpose(
                out=gdT_ps[:],
                in_=g_sb[:, C : C + 1],
                identity=ident[0:JP1, 0:JP1],
            )
            gd_sb = sb.tile([1, JP1], F32)
            nc.vector.tensor_copy(out=gd_sb[:], in_=gdT_ps[:])
            gb_ps = ps_small.tile([P, JP1], F32, tag="m")
            nc.tensor.matmul(gb_ps[:], ones_row[:], gd_sb[:], start=True, stop=True)
            dm = sb.tile([P, NCH, JP1], F32)
            gb_ap = gb_ps[:]
            gb_bc = type(gb_ap)(
                tensor=gb_ap.tensor,
                offset=gb_ap.offset,
                ap=[gb_ap.ap[0], [0, NCH], gb_ap.ap[1]],
            )
            nc.vector.tensor_mul(dm[:], wpj_sb[:], gb_bc)
            den_all = sb.tile([P, NCH], F32)
            nc.vector.reduce_sum(
                out=den_all[:], in_=dm[:], axis=mybir.AxisListType.X
            )
            rfull = sb.tile([P, NCH], F32)
            nc.vector.reciprocal(out=rfull[:], in_=den_all[:])

            # num[n, :] = Vandermonde @ G, one matmul per 128-position chunk;
            # fused 1/den scaling on alternating DVE/ACT into the staging
            # buffer; store every 8 slots
            o_big = sb.tile([P, NCH, C], F32)
            for s in range(NCH):
                num_ps = ps_num.tile([P, C], F32)
                nc.tensor.matmul(
                    num_ps[:],
                    wvs_sb[:, s * P : (s + 1) * P],
                    g_sb[:, 0:C],
                    start=True,
                    stop=True,
                )
                if s % 2 == 0:
                    nc.vector.tensor_scalar_mul(
                        o_big[:, s, :], num_ps[:], rfull[:, s : s + 1]
                    )
                else:
                    nc.scalar.activation(
                        out=o_big[:, s, :],
                        in_=num_ps[:],
                        func=mybir.ActivationFunctionType.Copy,
                        scale=rfull[:, s : s + 1],
                    )
                if s % 8 == 7:
                    s0 = s - 7
                    nc.sync.dma_start(
                        out=ov[:, s0 : s0 + 8, :], in_=o_big[:, s0 : s0 + 8, :]
                    )

    nc.compile()
    return nc


_nc_cache = None


def _get_nc():
    global _nc_cache
    if _nc_cache is None:
        _nc_cache = build_nc()
    return _nc_cache


def make_in_maps(x, wq, bq, wk, bk, wv, bv, wp, bp):
    f = lambda a: np.ascontiguousarray(np.asarray(a, dtype=np.float32))
    x = f(x)
    wv_, wp_, bv_, bp_ = f(wv), f(wp), f(bv), f(bp)
    shared = {
        "wqk1": (
            np.concatenate([f(wq).T @ f(wk), (f(bq) @ f(wk))[None, :]], 0)
            * np.float32(1.0 / A_S)
        ).astype(np.float32),
        "wvp1": np.concatenate(
            [
                np.concatenate(
                    [(wp_ @ wv_).T, (wp_ @ bv_ + bp_)[None, :]], 0
                ),
                np.eye(C + 1, dtype=np.float32)[:, C : C + 1],
            ],
            1,
        ).astype(np.float32),
        "bmat": BMAT,
        "wvs": WVS,
        "wpj": WPJ.reshape(P, NCH * JP1),
    }
    shared = {k: np.ascontiguousarray(v) for k, v in shared.items()}
    return [
        {"xb": np.ascontiguousarray(x[b].reshape(N, C)), **shared}
        for b in range(B)
    ]


def kernel_with_results(trace=False, **inputs):
    in_maps = make_in_maps(**inputs)
    nc = _get_nc()
    res = run_bass_kernel_spmd(nc, in_maps, core_ids=list(range(B)), trace=trace)
    out = np.stack([r["out"] for r in res.results], 0).reshape(B, H, W, C)
    return out, res


def kernel(**inputs):
    out, _ = kernel_with_results(**inputs)
    return out


# revision 36
# speedup vs baseline: 1.2618x; 1.0369x over previous
"""Trainium2 Bass kernel for nn_Attention_78048145703090 (sparse_attention).

Math: the reference's [N,N] attention is rank-1 structured: every logit row n
is W_n * s where s[m] = scale * (q_center . k_m) is one shared score vector
per sample and W_n = exp(1 - dist_n) * scale depends only on grid distance.
Instead of materializing E'[m,n] = exp(W_n * s_m) with 1.87M on-chip exps
(ACT-bound, ~26us in the original), the kernel uses a bivariate polynomial
fit (host-side least squares, f64):

    exp(W_n * A_S * y) ~= sum_{j<=J, k<=K} B[j,k] * W_n^j * y^k,  y = s/A_S

With P[m,k] = sn_m^k built by 4 log-depth DVE multiplies, the whole attention
collapses to small dense algebra:

    A^T  = [x|1]^T P                  (32 tiny PE matmuls, bf16)
    acw  = A^T^T [wv@wp ; bv@wp+bp | e]   (fold V+proj+biases; biases ride
                                           the ones-moment row)   [K+1, 65]
    G    = B acw                      [J+1, 65]  (tiny f32 matmul)
    num[n, :] = sum_j W_n^j G[j, 0:64]   -> 32 matmuls vs a host-constant
                                            f32 Vandermonde in W_n
    den[n]    = sum_j W_n^j G[j, 64]     -> one DVE mul+reduce+reciprocal
    out[n, :] = num[n, :] / den[n]       -> fused per-chunk scalar scaling

No exps, no softmax, no [N,N] or [N,U] intermediates, no transposes of the
output, and no gather: the Vandermonde stationary (230KB) replaces the 4MiB
one-hot matrix a previous version streamed. The W-direction evaluation must
stay f32 (exponential dynamic range -> bf16 cancellation fails); x and the
sn-powers are bf16 (validated 3.1e-4 end-to-end).

Sharding: data-parallel over B=8 across the 8 cores (one sample per core);
each core holds the full 64x64 weights.
"""

import os
import sys

sys.path.insert(0, "/opt/trn_rl_repo")

import numpy as np

import concourse.bacc as bacc
import concourse.mybir as mybir
import concourse.tile as tile
from concourse import masks
from concourse.tile_rust import add_dep_helper


def _install_profile_hook():
    """This image's antenv lacks axon_hooks; reconstruct it so
    run_bass_kernel_spmd(trace=True) can capture NTFF profiles. No-op for
    normal (untraced) runs."""
    import types

    try:
        import antenv.axon_hooks  # noqa: F401

        return
    except ImportError:
        pass
    try:
        import antenv

        m = types.ModuleType("antenv.axon_hooks")
        state = {"hook": None}
        m.set_axon_ntff_profile_hook = lambda h: state.__setitem__("hook", h)
        m.get_axon_ntff_profile_hook = lambda: state["hook"]
        sys.modules["antenv.axon_hooks"] = m
        antenv.axon_hooks = m
        from trn_agent_boot.trn_boot import _ntff_profile_via_ctypes

        m.set_axon_ntff_profile_hook(
            _ntff_profile_via_ctypes("/opt/axon/libaxon_pjrt.so")
        )
    except Exception:
        pass


_install_profile_hook()

from concourse.bass_utils import run_bass_kernel_spmd

B, H, W, C = 8, 64, 64, 64
N = H * W  # 4096
P = 128
NCH = N // P  # 32
HH = NCH // 2  # 16
CENTER = (H // 2) * W + (W // 2)  # 2080
C_CH = CENTER % NCH  # chunk (inner index) holding the center row: 0
C_PCOL = CENTER // NCH  # partition/column of the center row: 65
SCALE = float(C) ** -0.5
F32 = mybir.dt.float32
BF16 = mybir.dt.bfloat16

KDEG = int(os.environ.get("K_DEG", "12"))  # sn-polynomial degree
JDEG = int(os.environ.get("J_DEG", "12"))  # W-polynomial degree
KP1 = KDEG + 1
JP1 = JDEG + 1
A_S = 18.0  # |s| bound; sn = s / A_S in [-1, 1]

# ---- compile-time constants derived from the distance grid ----
_yy, _xx = np.mgrid[0:H, 0:W]
_d2 = ((_yy - H // 2) ** 2 + (_xx - W // 2) ** 2).reshape(-1)  # [N] int
_Wn = np.exp(1.0 - np.sqrt(_d2.astype(np.float64))) * SCALE  # [N] weights


def _build_B():
    """Least-squares bivariate fit exp(w*A_S*y) ~= sum B[j,k] w^j y^k over
    the actual unique w values x chebyshev y-grid, in f64."""
    wg = np.unique(_Wn)
    npts = 4 * KDEG + 16
    yg = np.cos(np.pi * (np.arange(npts) + 0.5) / npts)
    WW, YY = np.meshgrid(wg, yg, indexing="ij")
    F = np.exp(WW * A_S * YY).reshape(-1)
    basis = np.stack(
        [
            (WW.reshape(-1) ** j) * (YY.reshape(-1) ** k)
            for j in range(JP1)
            for k in range(KP1)
        ],
        1,
    )
    coef, *_ = np.linalg.lstsq(basis, F, rcond=None)
    return coef.reshape(JP1, KP1)  # B[j, k]


_B = _build_B()
BMAT = np.ascontiguousarray(_B.T.astype(np.float32))  # [KP1, JP1] stationary

# Vandermonde in W_n, f32. Stationary layout: column p of chunk s is
# n = p*32 + s (matches the [p, s, c] output staging); DVE layout wpj[p,s,j].
_wv = np.stack([_Wn**j for j in range(JP1)], 0).astype(np.float32)  # [JP1, N]
_n_of = (np.arange(P)[None, :] * NCH + np.arange(NCH)[:, None]).reshape(-1)
WVS = np.ascontiguousarray(_wv[:, _n_of])  # [JP1, 32*128], chunk-s blocks
WPJ = np.ascontiguousarray(
    _wv.T.reshape(P, NCH, JP1).astype(np.float32)
)  # [p, s, j] for n = p*32+s


def _bc(sl, reps):
    """Broadcast the innermost (size-1) dim of a sliced AP to `reps`."""
    return type(sl)(
        tensor=sl.tensor, offset=sl.offset, ap=[*sl.ap[:-1], [0, reps]]
    )


def build_nc():
    nc = bacc.Bacc("TRN2", target_bir_lowering=False, debug=False, num_devices=B)
    xb = nc.dram_tensor("xb", [N, C], F32, kind="ExternalInput")
    wqk1 = nc.dram_tensor("wqk1", [C + 1, C], F32, kind="ExternalInput")
    wvp1 = nc.dram_tensor("wvp1", [C + 1, C + 1], F32, kind="ExternalInput")
    bmat = nc.dram_tensor("bmat", [KP1, JP1], F32, kind="ExternalInput")
    wvs = nc.dram_tensor("wvs", [JP1, N], F32, kind="ExternalInput")
    wpj = nc.dram_tensor("wpj", [P, NCH * JP1], F32, kind="ExternalInput")
    out = nc.dram_tensor("out", [N, C], F32, kind="ExternalOutput")

    xv = xb.ap().rearrange("(p i) c -> p i c", p=P)
    ov = out.ap().rearrange("(p s) c -> p s c", p=P)

    with tile.TileContext(nc) as tc:
        with (
            tc.tile_pool(name="consts", bufs=1) as consts,
            tc.tile_pool(name="sb", bufs=1) as sb,
            tc.tile_pool(name="ps_t", bufs=2, space="PSUM") as ps_t,
            tc.tile_pool(name="ps_a", bufs=1, space="PSUM") as ps_a,
            tc.tile_pool(name="ps_small", bufs=2, space="PSUM") as ps_small,
            tc.tile_pool(name="ps_num", bufs=2, space="PSUM") as ps_num,
            tc.tile_pool(name="ps_w", bufs=1, space="PSUM") as ps_w,
        ):
            ident = consts.tile([P, P], F32)
            masks.make_identity(nc, ident[:])
            ones_row = consts.tile([1, P], F32)
            nc.vector.memset(ones_row[:], 1.0)

            # preload the ACT copy-table set while DMAs run
            actwarm = sb.tile([1, 2], F32)
            nc.scalar.copy(out=actwarm[:, 0:1], in_=ones_row[:, 0:1])

            # x on the sync HWDGE ring; Vandermonde consts follow it; small
            # weights on the scalar HWDGE ring (rings drain FIFO)
            x_sb = sb.tile([P, NCH, C], F32)
            xc0 = nc.sync.dma_start(out=x_sb[:, 0:1, :], in_=xv[:, 0:1, :])
            xha = nc.sync.dma_start(out=x_sb[:, 1:HH, :], in_=xv[:, 1:HH, :])
            xhb = nc.sync.dma_start(
                out=x_sb[:, HH:NCH, :], in_=xv[:, HH:NCH, :]
            )
            wvs_sb = consts.tile([JP1, N], F32)
            wvs_dma = nc.sync.dma_start(out=wvs_sb[:], in_=wvs[:])
            wpj_sb = consts.tile([P, NCH, JP1], F32)
            wpj_dma = nc.sync.dma_start(
                out=wpj_sb[:],
                in_=wpj.ap().rearrange("p (s j) -> p s j", s=NCH),
            )
            wqk1_sb = consts.tile([C + 1, C], F32)
            wq_dma = nc.scalar.dma_start(out=wqk1_sb[:], in_=wqk1[:])
            wvp1_sb = consts.tile([C + 1, C + 1], F32)
            wvp_dma = nc.scalar.dma_start(out=wvp1_sb[:], in_=wvp1[:])
            bmat_sb = consts.tile([KP1, JP1], F32)
            bmat_dma = nc.scalar.dma_start(out=bmat_sb[:], in_=bmat[:])
            for a, b_ in (
                (xha, xc0),
                (xhb, xha),
                (wvs_dma, xhb),
                (wpj_dma, wvs_dma),
                (wvp_dma, wq_dma),
                (bmat_dma, wvp_dma),
            ):
                add_dep_helper(a.ins, b_.ins, sync=False, reason="dma order")

            # bf16 ones-column copy of x, per half, on the scalar engine
            x1f = sb.tile([P, NCH, C + 1], BF16)
            nc.vector.memset(x1f[:, :, C : C + 1], 1.0)
            for h in range(2):
                i0 = h * HH
                nc.scalar.copy(
                    out=x1f[:, i0 : i0 + HH, 0:C], in_=x_sb[:, i0 : i0 + HH, :]
                )

            # q_center: transpose the center chunk, take the center column
            qcr_sb = sb.tile([C + 1, 1], F32)
            nc.vector.memset(qcr_sb[:], 1.0)
            xrow_ps = ps_small.tile([C, P], F32, tag="m")
            nc.tensor.transpose(
                out=xrow_ps[:], in_=x_sb[:, C_CH, :], identity=ident[:]
            )
            nc.vector.tensor_copy(
                out=qcr_sb[0:C, :], in_=xrow_ps[:, C_PCOL : C_PCOL + 1]
            )
            # u_row = qcr^T [wq.T wk ; bq wk] (pre-scaled by 1/A_S on host)
            ur_ps = ps_small.tile([1, C], F32, tag="m")
            nc.tensor.matmul(ur_ps[:], qcr_sb[:], wqk1_sb[:], start=True, stop=True)
            ur_sb = sb.tile([1, C], F32)
            nc.vector.tensor_copy(out=ur_sb[:], in_=ur_ps[:])
            ubc_ps = ps_small.tile([P, C], F32, tag="m")
            nc.tensor.matmul(ubc_ps[:], ones_row[:], ur_sb[:], start=True, stop=True)
            ubc_sb = sb.tile([P, C], F32)
            nc.vector.tensor_copy(out=ubc_sb[:], in_=ubc_ps[:])

            # sn[m] = x[m, :] . u  (already 1/A_S-scaled): mul + reduce per half
            s_col = sb.tile([P, NCH], F32)
            xu = [sb.tile([P, HH, C], F32, name=f"xu{h}") for h in range(2)]
            ubc_ap = ubc_sb[:]
            ubc_h = type(ubc_ap)(
                tensor=ubc_ap.tensor,
                offset=ubc_ap.offset,
                ap=[ubc_ap.ap[0], [0, HH], ubc_ap.ap[1]],
            )
            for h in range(2):
                i0 = h * HH
                nc.vector.tensor_mul(xu[h][:], x_sb[:, i0 : i0 + HH, :], ubc_h)
                nc.vector.reduce_sum(
                    out=s_col[:, i0 : i0 + HH],
                    in_=xu[h][:],
                    axis=mybir.AxisListType.X,
                )

            # PE warm-up burst: back-to-back matmuls over x while the DVE
            # computes sn, so the HAM clock gate opens (1.2 -> 2.4 GHz)
            # before the moment matmuls. Results are never read.
            warm_ps = ps_w.tile([C, 512], F32, tag="warm")
            x_ap = x_sb[:]
            x_flat = type(x_ap)(
                tensor=x_ap.tensor, offset=x_ap.offset, ap=[x_ap.ap[0], [1, 512]]
            )
            for wi in range(3):
                nc.tensor.matmul(
                    warm_ps[:],
                    x_sb[:, 0, :],
                    x_flat,
                    start=(wi == 0),
                    stop=(wi == 2),
                )

            # monomial basis S[:, i, k] = sn^k via log-depth squaring (f32)
            S = sb.tile([P, NCH, KP1], F32)
            nc.vector.memset(S[:, :, 0:1], 1.0)
            nc.vector.tensor_copy(out=S[:, :, 1:2], in_=s_col[:])
            nc.vector.tensor_mul(S[:, :, 2:3], S[:, :, 1:2], S[:, :, 1:2])
            nc.vector.tensor_mul(S[:, :, 3:5], S[:, :, 1:3], _bc(S[:, :, 2:3], 2))
            nc.vector.tensor_mul(S[:, :, 5:9], S[:, :, 1:5], _bc(S[:, :, 4:5], 4))
            nc.vector.tensor_mul(
                S[:, :, 9:KP1], S[:, :, 5 : KP1 - 4], _bc(S[:, :, 4:5], 4)
            )
            Sb = sb.tile([P, NCH, KP1], BF16)
            nc.vector.tensor_copy(out=Sb[:], in_=S[:])

            # moments A^T[c1, k] = sum_m [x|1] sn^k: 32 accumulating matmuls
            at_ps = ps_a.tile([C + 1, KP1], F32)
            for i in range(NCH):
                nc.tensor.matmul(
                    at_ps[:],
                    x1f[:, i, :],
                    Sb[:, i, :],
                    start=(i == 0),
                    stop=(i == NCH - 1),
                )
            at_sb = sb.tile([C + 1, KP1], F32)
            nc.vector.tensor_copy(out=at_sb[:], in_=at_ps[:])

            # acw = A^T^T [CW|e64] (fold wv/wp/biases), then G = B acw
            acw_ps = ps_small.tile([KP1, C + 1], F32, tag="m")
            nc.tensor.matmul(acw_ps[:], at_sb[:], wvp1_sb[:], start=True, stop=True)
            acw_sb = sb.tile([KP1, C + 1], F32)
            nc.vector.tensor_copy(out=acw_sb[:], in_=acw_ps[:])
            g_ps = ps_small.tile([JP1, C + 1], F32, tag="m")
            nc.tensor.matmul(g_ps[:], bmat_sb[:], acw_sb[:], start=True, stop=True)
            g_sb = sb.tile([JP1, C + 1], F32)
            nc.vector.tensor_copy(out=g_sb[:], in_=g_ps[:])

            # den coefficients: transpose G col 64 -> row, broadcast across
            # partitions, then den/reciprocal per position on the DVE
            gdT_ps = ps_small.tile([1, JP1], F32, tag="m")
            nc.tensor.transpose(
                out=gdT_ps[:],
                in_=g_sb[:, C : C + 1],
                identity=ident[0:JP1, 0:JP1],
            )
            gd_sb = sb.tile([1, JP1], F32)
            nc.vector.tensor_copy(out=gd_sb[:], in_=gdT_ps[:])
            gb_ps = ps_small.tile([P, JP1], F32, tag="m")
            nc.tensor.matmul(gb_ps[:], ones_row[:], gd_sb[:], start=True, stop=True)
            dm = sb.tile([P, NCH, JP1], F32)
            gb_ap = gb_ps[:]
            gb_bc = type(gb_ap)(
                tensor=gb_ap.tensor,
                offset=gb_ap.offset,
                ap=[gb_ap.ap[0], [0, NCH], gb_ap.ap[1]],
            )
            nc.vector.tensor_mul(dm[:], wpj_sb[:], gb_bc)
            den_all = sb.tile([P, NCH], F32)
            nc.vector.reduce_sum(
                out=den_all[:], in_=dm[:], axis=mybir.AxisListType.X
            )
            rfull = sb.tile([P, NCH], F32)
            nc.vector.reciprocal(out=rfull[:], in_=den_all[:])

            # num[n, :] = Vandermonde @ G, one matmul per 128-position chunk
            # into an 8-chunk grouped PSUM tile; one wide DVE tensor_tensor
            # per group applies 1/den (rfull broadcast along c); store per
            # group. Stationaries alternate row groups 0/32 for LDW overlap.
            o_big = sb.tile([P, NCH, C], F32)
            GRP = 8
            for g0 in range(0, NCH, GRP):
                num_ps = ps_num.tile([P, GRP, C], F32)
                for k in range(GRP):
                    s = g0 + k
                    nc.tensor.matmul(
                        num_ps[:, k, :],
                        wvs_sb[:, s * P : (s + 1) * P],
                        g_sb[:, 0:C],
                        start=True,
                        stop=True,
                    )
                r_ap = rfull[:, g0 : g0 + GRP]
                r_bc = type(r_ap)(
                    tensor=r_ap.tensor,
                    offset=r_ap.offset,
                    ap=[r_ap.ap[0], r_ap.ap[1], [0, C]],
                )
                nc.vector.tensor_mul(
                    o_big[:, g0 : g0 + GRP, :], num_ps[:], r_bc
                )
                nc.sync.dma_start(
                    out=ov[:, g0 : g0 + GRP, :], in_=o_big[:, g0 : g0 + GRP, :]
                )

    nc.compile()
    return nc


_nc_cache = None


def _get_nc():
    global _nc_cache
    if _nc_cache is None:
        _nc_cache = build_nc()
    return _nc_cache


def make_in_maps(x, wq, bq, wk, bk, wv, bv, wp, bp):
    f = lambda a: np.ascontiguousarray(np.asarray(a, dtype=np.float32))
    x = f(x)
    wv_, wp_, bv_, bp_ = f(wv), f(wp), f(bv), f(bp)
    shared = {
        "wqk1": (
            np.concatenate([f(wq).T @ f(wk), (f(bq) @ f(wk))[None, :]], 0)
            * np.float32(1.0 / A_S)
        ).astype(np.float32),
        "wvp1": np.concatenate(
            [
                np.concatenate(
                    [(wp_ @ wv_).T, (wp_ @ bv_ + bp_)[None, :]], 0
                ),
                np.eye(C + 1, dtype=np.float32)[:, C : C + 1],
            ],
            1,
        ).astype(np.float32),
        "bmat": BMAT,
        "wvs": WVS,
        "wpj": WPJ.reshape(P, NCH * JP1),
    }
    shared = {k: np.ascontiguousarray(v) for k, v in shared.items()}
    return [
        {"xb": np.ascontiguousarray(x[b].reshape(N, C)), **shared}
        for b in range(B)
    ]


def kernel_with_results(trace=False, **inputs):
    in_maps = make_in_maps(**inputs)
    nc = _get_nc()
    res = run_bass_kernel_spmd(nc, in_maps, core_ids=list(range(B)), trace=trace)
    out = np.stack([r["out"] for r in res.results], 0).reshape(B, H, W, C)
    return out, res


def kernel(**inputs):
    out, _ = kernel_with_results(**inputs)
    return out


# revision 45
# speedup vs baseline: 1.5952x; 1.2642x over previous
"""Trainium2 Bass kernel for nn_Attention_78048145703090 (sparse_attention).

Math: the reference's [N,N] attention is rank-1 structured: every logit row n
is W_n * s where s[m] = scale * (q_center . k_m) is one shared score vector
per sample and W_n = exp(1 - dist_n) * scale depends only on grid distance.
Instead of materializing E'[m,n] = exp(W_n * s_m) with 1.87M on-chip exps
(ACT-bound, ~26us in the original), the kernel uses a bivariate polynomial
fit (host-side least squares, f64):

    exp(W_n * A_S * y) ~= sum_{j<=J, k<=K} B[j,k] * W_n^j * y^k,  y = s/A_S

With P[m,k] = sn_m^k built by 4 log-depth DVE multiplies, the whole attention
collapses to small dense algebra:

    A^T  = [x|1]^T P                  (32 tiny PE matmuls, bf16)
    acw  = A^T^T [wv@wp ; bv@wp+bp | e]   (fold V+proj+biases; biases ride
                                           the ones-moment row)   [K+1, 65]
    G    = B acw                      [J+1, 65]  (tiny f32 matmul)
    num[n, :] = sum_j W_n^j G[j, 0:64]   -> 32 matmuls vs a host-constant
                                            f32 Vandermonde in W_n
    den[n]    = sum_j W_n^j G[j, 64]     -> one DVE mul+reduce+reciprocal
    out[n, :] = num[n, :] / den[n]       -> fused per-chunk scalar scaling

No exps, no softmax, no [N,N] or [N,U] intermediates, no transposes of the
output, and no gather: the Vandermonde stationary (230KB) replaces the 4MiB
one-hot matrix a previous version streamed. The W-direction evaluation must
stay f32 (exponential dynamic range -> bf16 cancellation fails); x and the
sn-powers are bf16 (validated 3.1e-4 end-to-end).

Sharding: data-parallel over B=8 across the 8 cores (one sample per core);
each core holds the full 64x64 weights.
"""

import os
import sys

sys.path.insert(0, "/opt/trn_rl_repo")

import numpy as np

import concourse.bacc as bacc
import concourse.mybir as mybir
import concourse.tile as tile
from concourse import masks
from concourse.tile_rust import add_dep_helper


def _install_profile_hook():
    """This image's antenv lacks axon_hooks; reconstruct it so
    run_bass_kernel_spmd(trace=True) can capture NTFF profiles. No-op for
    normal (untraced) runs."""
    import types

    try:
        import antenv.axon_hooks  # noqa: F401

        return
    except ImportError:
        pass
    try:
        import antenv

        m = types.ModuleType("antenv.axon_hooks")
        state = {"hook": None}
        m.set_axon_ntff_profile_hook = lambda h: state.__setitem__("hook", h)
        m.get_axon_ntff_profile_hook = lambda: state["hook"]
        sys.modules["antenv.axon_hooks"] = m
        antenv.axon_hooks = m
        from trn_agent_boot.trn_boot import _ntff_profile_via_ctypes

        m.set_axon_ntff_profile_hook(
            _ntff_profile_via_ctypes("/opt/axon/libaxon_pjrt.so")
        )
    except Exception:
        pass


_install_profile_hook()

from concourse.bass_utils import run_bass_kernel_spmd

B, H, W, C = 8, 64, 64, 64
N = H * W  # 4096
P = 128
NCH = N // P  # 32
HH = NCH // 2  # 16
CENTER = (H // 2) * W + (W // 2)  # 2080
C_CH = CENTER % NCH  # chunk (inner index) holding the center row: 0
C_PCOL = CENTER // NCH  # partition/column of the center row: 65
SCALE = float(C) ** -0.5
F32 = mybir.dt.float32
BF16 = mybir.dt.bfloat16

KDEG = int(os.environ.get("K_DEG", "12"))  # sn-polynomial degree
JDEG = int(os.environ.get("J_DEG", "12"))  # W-polynomial degree
KP1 = KDEG + 1
JP1 = JDEG + 1
A_S = 18.0  # |s| bound; sn = s / A_S in [-1, 1]

# ---- compile-time constants derived from the distance grid ----
_yy, _xx = np.mgrid[0:H, 0:W]
_d2 = ((_yy - H // 2) ** 2 + (_xx - W // 2) ** 2).reshape(-1)  # [N] int
_Wn = np.exp(1.0 - np.sqrt(_d2.astype(np.float64))) * SCALE  # [N] weights


_WMAX = _Wn.max()


def _wtil(w):
    return 2.0 * w / _WMAX - 1.0  # w -> [-1, 1]


def _build_B():
    """Least-squares bivariate fit exp(w*A_S*y) ~= sum B[j,k] T_j(wt) y^k
    over the actual unique w values x chebyshev y-grid, in f64. The
    Chebyshev-in-w basis keeps the coefficients bounded so the W-direction
    evaluation survives bf16 (raw w^j monomials do not)."""
    from numpy.polynomial import chebyshev as Ch

    wg = np.unique(_Wn)
    npts = 4 * KDEG + 16
    yg = np.cos(np.pi * (np.arange(npts) + 0.5) / npts)
    WW, YY = np.meshgrid(wg, yg, indexing="ij")
    F = np.exp(WW * A_S * YY).reshape(-1)
    eyeJ = np.eye(JP1)
    Tj = np.stack(
        [Ch.chebval(_wtil(WW.reshape(-1)), eyeJ[j]) for j in range(JP1)], 1
    )
    basis = np.stack(
        [Tj[:, j] * (YY.reshape(-1) ** k) for j in range(JP1) for k in range(KP1)],
        1,
    )
    coef, *_ = np.linalg.lstsq(basis, F, rcond=None)
    return coef.reshape(JP1, KP1)  # B[j, k]


_B = _build_B()
BMAT = np.ascontiguousarray(_B.T.astype(np.float32))  # [KP1, JP1] stationary

import ml_dtypes

# Chebyshev-in-w design matrix T_j(wt(W_n)), bf16. Stationary layout: column
# p of chunk s is n = p*32 + s (matches the [p, s, c] output staging); DVE
# layout wpj[p, s, j] for the den evaluation.
from numpy.polynomial import chebyshev as _chb

_eyeJ = np.eye(JP1)
_wv = np.stack(
    [_chb.chebval(_wtil(_Wn), _eyeJ[j]) for j in range(JP1)], 0
).astype(np.float32)  # [JP1, N]
_n_of = (np.arange(P)[None, :] * NCH + np.arange(NCH)[:, None]).reshape(-1)
WVS = np.ascontiguousarray(_wv[:, _n_of].astype(ml_dtypes.bfloat16))
WPJ = np.ascontiguousarray(
    _wv.T.reshape(P, NCH, JP1).astype(ml_dtypes.bfloat16)
)  # [p, s, j] for n = p*32+s


def _bc(sl, reps):
    """Broadcast the innermost (size-1) dim of a sliced AP to `reps`."""
    return type(sl)(
        tensor=sl.tensor, offset=sl.offset, ap=[*sl.ap[:-1], [0, reps]]
    )


def build_nc():
    nc = bacc.Bacc("TRN2", target_bir_lowering=False, debug=False, num_devices=B)
    xb = nc.dram_tensor("xb", [N, C], F32, kind="ExternalInput")
    wqk1 = nc.dram_tensor("wqk1", [C + 1, C], F32, kind="ExternalInput")
    wvp1 = nc.dram_tensor("wvp1", [C + 1, C + 1], F32, kind="ExternalInput")
    bmat = nc.dram_tensor("bmat", [KP1, JP1], F32, kind="ExternalInput")
    wvs = nc.dram_tensor("wvs", [JP1, N], BF16, kind="ExternalInput")
    wpj = nc.dram_tensor("wpj", [P, NCH * JP1], BF16, kind="ExternalInput")
    out = nc.dram_tensor("out", [N, C], F32, kind="ExternalOutput")

    xv = xb.ap().rearrange("(p i) c -> p i c", p=P)
    ov = out.ap().rearrange("(p s) c -> p s c", p=P)

    with tile.TileContext(nc) as tc:
        with (
            tc.tile_pool(name="consts", bufs=1) as consts,
            tc.tile_pool(name="sb", bufs=1) as sb,
            tc.tile_pool(name="ps_t", bufs=2, space="PSUM") as ps_t,
            tc.tile_pool(name="ps_a", bufs=1, space="PSUM") as ps_a,
            tc.tile_pool(name="ps_small", bufs=2, space="PSUM") as ps_small,
            tc.tile_pool(name="ps_num", bufs=2, space="PSUM") as ps_num,
            tc.tile_pool(name="ps_w", bufs=1, space="PSUM") as ps_w,
        ):
            ident = consts.tile([P, P], F32)
            masks.make_identity(nc, ident[:])
            ones_row = consts.tile([1, P], F32)
            nc.vector.memset(ones_row[:], 1.0)

            # preload the ACT copy-table set while DMAs run
            actwarm = sb.tile([1, 2], F32)
            nc.scalar.copy(out=actwarm[:, 0:1], in_=ones_row[:, 0:1])

            # x on the sync HWDGE ring; Vandermonde consts follow it; small
            # weights on the scalar HWDGE ring (rings drain FIFO)
            x_sb = sb.tile([P, NCH, C], F32)
            xc0 = nc.sync.dma_start(out=x_sb[:, 0:1, :], in_=xv[:, 0:1, :])
            xha = nc.sync.dma_start(out=x_sb[:, 1:HH, :], in_=xv[:, 1:HH, :])
            xhb = nc.scalar.dma_start(
                out=x_sb[:, HH:NCH, :], in_=xv[:, HH:NCH, :]
            )
            wvs_sb = consts.tile([JP1, N], BF16)
            wvs_dma = nc.sync.dma_start(out=wvs_sb[:], in_=wvs[:])
            wpj_sb = consts.tile([P, NCH, JP1], BF16)
            wpj_dma = nc.sync.dma_start(
                out=wpj_sb[:],
                in_=wpj.ap().rearrange("p (s j) -> p s j", s=NCH),
            )
            wqk1_sb = consts.tile([C + 1, C], F32)
            wq_dma = nc.scalar.dma_start(out=wqk1_sb[:], in_=wqk1[:])
            wvp1_sb = consts.tile([C + 1, C + 1], F32)
            wvp_dma = nc.scalar.dma_start(out=wvp1_sb[:], in_=wvp1[:])
            bmat_sb = consts.tile([KP1, JP1], F32)
            bmat_dma = nc.scalar.dma_start(out=bmat_sb[:], in_=bmat[:])
            for a, b_ in (
                (xha, xc0),
                (wvs_dma, xha),
                (wpj_dma, wvs_dma),
                (wvp_dma, wq_dma),
                (bmat_dma, wvp_dma),
                (xhb, bmat_dma),
            ):
                add_dep_helper(a.ins, b_.ins, sync=False, reason="dma order")

            # bf16 ones-column copy of x, per half, on the scalar engine
            x1f = sb.tile([P, NCH, C + 1], BF16)
            nc.vector.memset(x1f[:, :, C : C + 1], 1.0)
            for h in range(2):
                i0 = h * HH
                nc.scalar.copy(
                    out=x1f[:, i0 : i0 + HH, 0:C], in_=x_sb[:, i0 : i0 + HH, :]
                )

            # q_center: transpose the center chunk, take the center column
            qcr_sb = sb.tile([C + 1, 1], F32)
            nc.vector.memset(qcr_sb[:], 1.0)
            xrow_ps = ps_small.tile([C, P], F32, tag="m")
            nc.tensor.transpose(
                out=xrow_ps[:], in_=x_sb[:, C_CH, :], identity=ident[:]
            )
            nc.vector.tensor_copy(
                out=qcr_sb[0:C, :], in_=xrow_ps[:, C_PCOL : C_PCOL + 1]
            )
            # u_row = qcr^T [wq.T wk ; bq wk] (pre-scaled by 1/A_S on host)
            ur_ps = ps_small.tile([1, C], F32, tag="m")
            nc.tensor.matmul(ur_ps[:], qcr_sb[:], wqk1_sb[:], start=True, stop=True)
            ur_sb = sb.tile([1, C], F32)
            nc.vector.tensor_copy(out=ur_sb[:], in_=ur_ps[:])
            ubc_ps = ps_small.tile([P, C], F32, tag="m")
            nc.tensor.matmul(ubc_ps[:], ones_row[:], ur_sb[:], start=True, stop=True)
            ubc_sb = sb.tile([P, C], F32)
            nc.vector.tensor_copy(out=ubc_sb[:], in_=ubc_ps[:])

            # sn[m] = x[m, :] . u  (already 1/A_S-scaled): mul + reduce per half
            s_col = sb.tile([P, NCH], F32)
            xu = [sb.tile([P, HH, C], F32, name=f"xu{h}") for h in range(2)]
            ubc_ap = ubc_sb[:]
            ubc_h = type(ubc_ap)(
                tensor=ubc_ap.tensor,
                offset=ubc_ap.offset,
                ap=[ubc_ap.ap[0], [0, HH], ubc_ap.ap[1]],
            )
            for h in range(2):
                i0 = h * HH
                nc.vector.tensor_mul(xu[h][:], x_sb[:, i0 : i0 + HH, :], ubc_h)
                nc.vector.reduce_sum(
                    out=s_col[:, i0 : i0 + HH],
                    in_=xu[h][:],
                    axis=mybir.AxisListType.X,
                )

            # PE warm-up burst: back-to-back matmuls over x while the DVE
            # computes sn, so the HAM clock gate opens (1.2 -> 2.4 GHz)
            # before the moment matmuls. Results are never read.
            warm_ps = ps_w.tile([C, 512], F32, tag="warm")
            x_ap = x_sb[:]
            x_flat = type(x_ap)(
                tensor=x_ap.tensor, offset=x_ap.offset, ap=[x_ap.ap[0], [1, 512]]
            )
            for wi in range(3):
                nc.tensor.matmul(
                    warm_ps[:],
                    x_sb[:, 0, :],
                    x_flat,
                    start=(wi == 0),
                    stop=(wi == 2),
                )

            # monomial basis S[:, i, k] = sn^k via log-depth squaring (f32)
            S = sb.tile([P, NCH, KP1], F32)
            nc.vector.memset(S[:, :, 0:1], 1.0)
            nc.vector.tensor_copy(out=S[:, :, 1:2], in_=s_col[:])
            nc.vector.tensor_mul(S[:, :, 2:3], S[:, :, 1:2], S[:, :, 1:2])
            nc.vector.tensor_mul(S[:, :, 3:5], S[:, :, 1:3], _bc(S[:, :, 2:3], 2))
            nc.vector.tensor_mul(S[:, :, 5:9], S[:, :, 1:5], _bc(S[:, :, 4:5], 4))
            nc.vector.tensor_mul(
                S[:, :, 9:KP1], S[:, :, 5 : KP1 - 4], _bc(S[:, :, 4:5], 4)
            )
            Sb = sb.tile([P, NCH, KP1], BF16)
            nc.vector.tensor_copy(out=Sb[:], in_=S[:])

            # moments A^T[c1, k] = sum_m [x|1] sn^k: 32 accumulating matmuls
            at_ps = ps_a.tile([C + 1, KP1], F32)
            for i in range(NCH):
                nc.tensor.matmul(
                    at_ps[:],
                    x1f[:, i, :],
                    Sb[:, i, :],
                    start=(i == 0),
                    stop=(i == NCH - 1),
                )
            at_sb = sb.tile([C + 1, KP1], F32)
            nc.vector.tensor_copy(out=at_sb[:], in_=at_ps[:])

            # acw = A^T^T [CW|e64] (fold wv/wp/biases), then G = B acw
            acw_ps = ps_small.tile([KP1, C + 1], F32, tag="m")
            nc.tensor.matmul(acw_ps[:], at_sb[:], wvp1_sb[:], start=True, stop=True)
            acw_sb = sb.tile([KP1, C + 1], F32)
            nc.vector.tensor_copy(out=acw_sb[:], in_=acw_ps[:])
            g_ps = ps_small.tile([JP1, C + 1], F32, tag="m")
            nc.tensor.matmul(g_ps[:], bmat_sb[:], acw_sb[:], start=True, stop=True)
            g_sb = sb.tile([JP1, C + 1], F32)
            nc.vector.tensor_copy(out=g_sb[:], in_=g_ps[:])
            gbf_sb = sb.tile([JP1, C], BF16)
            nc.vector.tensor_copy(out=gbf_sb[:], in_=g_ps[:, 0:C])

            # den coefficients: transpose G col 64 -> row, broadcast across
            # partitions, then den/reciprocal per position on the DVE
            gdT_ps = ps_small.tile([1, JP1], F32, tag="m")
            nc.tensor.transpose(
                out=gdT_ps[:],
                in_=g_sb[:, C : C + 1],
                identity=ident[0:JP1, 0:JP1],
            )
            gd_sb = sb.tile([1, JP1], F32)
            nc.vector.tensor_copy(out=gd_sb[:], in_=gdT_ps[:])
            gb_ps = ps_small.tile([P, JP1], F32, tag="m")
            nc.tensor.matmul(gb_ps[:], ones_row[:], gd_sb[:], start=True, stop=True)
            gb_sb = sb.tile([P, JP1], BF16)
            nc.vector.tensor_copy(out=gb_sb[:], in_=gb_ps[:])
            dm = sb.tile([P, NCH, JP1], BF16)
            gb_ap = gb_sb[:]
            gb_bc = type(gb_ap)(
                tensor=gb_ap.tensor,
                offset=gb_ap.offset,
                ap=[gb_ap.ap[0], [0, NCH], gb_ap.ap[1]],
            )
            nc.vector.tensor_mul(dm[:], wpj_sb[:], gb_bc)
            den_all = sb.tile([P, NCH], F32)
            nc.vector.reduce_sum(
                out=den_all[:], in_=dm[:], axis=mybir.AxisListType.X
            )
            rfull = sb.tile([P, NCH], F32)
            nc.vector.reciprocal(out=rfull[:], in_=den_all[:])

            # num[n, :] = Vandermonde @ G, one matmul per 128-position chunk
            # into an 8-chunk grouped PSUM tile; one wide DVE tensor_tensor
            # per group applies 1/den (rfull broadcast along c); store per
            # group. Stationaries alternate row groups 0/32 for LDW overlap.
            o_big = sb.tile([P, NCH, C], F32)
            GRP = 8
            for g0 in range(0, NCH, GRP):
                num_ps = ps_num.tile([P, GRP, C], F32)
                for k in range(GRP):
                    s = g0 + k
                    nc.tensor.matmul(
                        num_ps[:, k, :],
                        wvs_sb[:, s * P : (s + 1) * P],
                        gbf_sb[:],
                        start=True,
                        stop=True,
                    )
                r_ap = rfull[:, g0 : g0 + GRP]
                r_bc = type(r_ap)(
                    tensor=r_ap.tensor,
                    offset=r_ap.offset,
                    ap=[r_ap.ap[0], r_ap.ap[1], [0, C]],
                )
                nc.vector.tensor_mul(
                    o_big[:, g0 : g0 + GRP, :], num_ps[:], r_bc
                )
                nc.sync.dma_start(
                    out=ov[:, g0 : g0 + GRP, :], in_=o_big[:, g0 : g0 + GRP, :]
                )

    nc.compile()
    return nc


_nc_cache = None


def _get_nc():
    global _nc_cache
    if _nc_cache is None:
        _nc_cache = build_nc()
    return _nc_cache


def make_in_maps(x, wq, bq, wk, bk, wv, bv, wp, bp):
    f = lambda a: np.ascontiguousarray(np.asarray(a, dtype=np.float32))
    x = f(x)
    wv_, wp_, bv_, bp_ = f(wv), f(wp), f(bv), f(bp)
    shared = {
        "wqk1": (
            np.concatenate([f(wq).T @ f(wk), (f(bq) @ f(wk))[None, :]], 0)
            * np.float32(1.0 / A_S)
        ).astype(np.float32),
        "wvp1": np.concatenate(
            [
                np.concatenate(
                    [(wp_ @ wv_).T, (wp_ @ bv_ + bp_)[None, :]], 0
                ),
                np.eye(C + 1, dtype=np.float32)[:, C : C + 1],
            ],
            1,
        ).astype(np.float32),
        "bmat": BMAT,
        "wvs": WVS,
        "wpj": np.ascontiguousarray(WPJ.reshape(P, NCH * JP1)),
    }
    shared = {k: np.ascontiguousarray(v) for k, v in shared.items()}
    return [
        {"xb": np.ascontiguousarray(x[b].reshape(N, C)), **shared}
        for b in range(B)
    ]


def kernel_with_results(trace=False, **inputs):
    in_maps = make_in_maps(**inputs)
    nc = _get_nc()
    res = run_bass_kernel_spmd(nc, in_maps, core_ids=list(range(B)), trace=trace)
    out = np.stack([r["out"] for r in res.results], 0).reshape(B, H, W, C)
    return out, res


def kernel(**inputs):
    out, _ = kernel_with_results(**inputs)
    return out
